# revision 1
# baseline (speedup 1.0000x reference)
"""Trainium2 Bass kernel for nn_ContrastiveLoss (NT-Xent with sampled negatives).

Reference semantics (B=4096, D=512, N=8192, R=4 negatives/row, temp=0.5+1e-8):
    z  = concat(z_i, z_j)                       [N, D]
    zn = z / max(||z||, 1e-8)
    sim = (zn @ zn.T) / temp
    pos[i]  = sim[i, (i+B) % N]
    cols    = neg_idx + (neg_idx >= row)        (skip-diagonal remap)
    neg[i,k] = sim[i, cols[i,k]]
    nll = logsumexp([pos, neg]) - pos ;  loss = mean(nll)

Key insight: only 5 entries of each sim row are needed, so we never form the
[N, N] matrix. Each of the 8 cores takes a 1024-row slab, gathers the 5
partner rows per row (1 static positive slab + 4 indirect-DMA gathers),
computes cosine dots with fused DVE tensor_tensor_reduce ops, norms with
fused ACT square+accum, then a 5-wide log-softmax and a partial sum.
Host sums the 8 partials.
"""

import os
import sys

import numpy as np

if "/opt/trn_rl_repo" not in sys.path:
    sys.path.insert(0, "/opt/trn_rl_repo")

B = 4096
D = 512
N = 2 * B
R = 4  # negatives per row
NCORES = 8
RPC = N // NCORES  # rows per core = 1024
P = 128  # partitions
J = RPC // P  # row-tiles per core = 8
TEMP = 0.5 + 1e-08
EPS = 1e-08
INV_TEMP = float(1.0 / TEMP)

_CACHE = {}


def build_nc():
    import concourse.bass as bass
    import concourse.bacc as bacc
    import concourse.mybir as mybir
    from concourse.tile import TileContext

    fp32 = mybir.dt.float32
    i32 = mybir.dt.int32

    # Bacc (not raw Bass): its compile pipeline legalizes TRN2's
    # one-sync-wait-per-instruction constraint via event semaphores.
    nc = bacc.Bacc()
    z_full = nc.dram_tensor("z_full", [N, D], fp32, kind="ExternalInput")
    # own rows followed by positive-partner rows, one DMA -> one wait
    zop = nc.dram_tensor("zop", [2 * RPC, D], fp32, kind="ExternalInput")
    # neg indices [P, R, J] followed by row ids [P, 1, J]
    idx = nc.dram_tensor("idx", [P, R + 1, J], i32, kind="ExternalInput")
    out_partial = nc.dram_tensor("partial", [1, 1], fp32, kind="ExternalOutput")
    dbg = os.environ.get("K_DEBUG", "0") == "1"
    if dbg:
        out_logit = nc.dram_tensor(
            "logit_out", [P, J, 1 + R], fp32, kind="ExternalOutput"
        )
        out_cols = nc.dram_tensor("cols_out", [P, R, J], i32, kind="ExternalOutput")
        out_g = nc.dram_tensor("g_out", [P, R * J * D], fp32, kind="ExternalOutput")

    AF = mybir.ActivationFunctionType
    OP = mybir.AluOpType

    with TileContext(nc) as tc:
        with (
            tc.tile_pool(name="big", bufs=1) as big,
            tc.tile_pool(name="small", bufs=1) as small,
            tc.tile_pool(name="scr", bufs=6) as scr,
            tc.tile_pool(name="psum", bufs=4, space="PSUM") as pp,
        ):
            # ---- bulk load (own rows + positive-partner rows, single DMA) ----
            ap_t = big.tile([P, 2 * J, D], fp32, tag="AP")
            # row r_local = t*128 + p  ->  tile[p, t, :]
            nc.sync.dma_start(
                out=ap_t[:], in_=zop[:].rearrange("(t p) d -> p t d", p=P)
            )
            a_t = ap_t[:, 0:J, :]
            p_t = ap_t[:, J : 2 * J, :]

            # ---- index prep: cols = neg + (neg >= row), laid out [P, R, J] so
            # each k-slice is contiguous for the indirect-DMA offset AP ----
            nr = small.tile([P, R + 1, J], i32, tag="nr")
            nc.sync.dma_start(out=nr[:], in_=idx[:])
            ni = nr[:, 0:R, :]
            ri = nr[:, R : R + 1, :]
            ge = small.tile([P, R, J], i32, tag="ge")
            cols = small.tile([P, R, J], i32, tag="cols")
            nc.vector.tensor_tensor(
                out=ge[:], in0=ni, in1=ri.to_broadcast([P, R, J]), op=OP.is_ge
            )
            nc.vector.tensor_tensor(out=cols[:], in0=ni, in1=ge[:], op=OP.add)

            # ---- negative-row gathers: HW indirect DMA honors ONE index per
            # dest partition row (multi-index-per-partition is sim-only), so
            # issue one [P,1]-index gather per (k, j): 32 DMAs of 128 rows ----
            g_all = big.tile([P, R, J, D], fp32, tag="G")
            for k in range(R):
                for j in range(J):
                    nc.gpsimd.indirect_dma_start(
                        out=g_all[:, k, j, :],
                        out_offset=None,
                        in_=z_full[:],
                        in_offset=bass.IndirectOffsetOnAxis(
                            ap=cols[:, k, j : j + 1], axis=0
                        ),
                    )
            g_t = [g_all[:, k, :, :] for k in range(R)]

            # ---- row sum-of-squares (ACT square with fused row-sum) ----
            ssa = small.tile([P, J, 1], fp32, tag="ssa")
            ssp = small.tile([P, J, 1], fp32, tag="ssp")
            ssg = small.tile([P, J, R], fp32, tag="ssg")
            for j in range(J):
                sq = scr.tile([P, D], fp32, tag="sq")
                nc.scalar.activation(
                    out=sq[:], in_=a_t[:, j, :], func=AF.Square,
                    accum_out=ssa[:, j, :],
                )
                sq = scr.tile([P, D], fp32, tag="sq")
                nc.scalar.activation(
                    out=sq[:], in_=p_t[:, j, :], func=AF.Square,
                    accum_out=ssp[:, j, :],
                )
                for k in range(R):
                    sq = scr.tile([P, D], fp32, tag="sq")
                    nc.scalar.activation(
                        out=sq[:], in_=g_t[k][:, j, :], func=AF.Square,
                        accum_out=ssg[:, j, k : k + 1],
                    )

            # ---- dots: one wide DVE multiply + one wide reduce per partner ----
            # (tensor_tensor_reduce is rejected by this walrus build, so
            # separate mult+reduce; wide [P, J*D] ops amortize issue overhead)
            from concourse.tile_rust import add_dep_helper

            # The TT ISA encoding has a single sync-wait slot, so each DVE
            # multiply may carry at most one semaphore wait: pin DVE order
            # (add_dep_helper) and give every partner its own product slot,
            # split in J-halves so slot reuse pairs only with an
            # already-observed DMA sem.
            J2 = J // 2
            dp = small.tile([P, J, 1], fp32, tag="dp")
            dg = small.tile([P, J, R], fp32, tag="dg")
            prev = None
            pairs = [(p_t, dp[:, :, 0:1], "pp")] + [
                (g_t[k], dg[:, :, k : k + 1], f"g{k}") for k in range(R)
            ]
            for x_ap, d_out, tag in pairs:
                for h in range(2):
                    js = slice(h * J2, (h + 1) * J2)
                    prod = big.tile([P, J2, D], fp32, tag=f"prod_{tag}")
                    mm = nc.vector.tensor_tensor(
                        out=prod[:], in0=a_t[:, js, :], in1=x_ap[:, js, :],
                        op=OP.mult,
                    )
                    if prev is not None:
                        add_dep_helper(mm.ins, prev.ins, sync=False,
                                       reason="dve-order")
                    prev = mm
                    nc.vector.tensor_reduce(
                        out=d_out[:, js, :], in_=prod[:],
                        axis=mybir.AxisListType.X, op=OP.add,
                    )

            # ---- inverse norms: inv = 1/max(sqrt(ss), eps) ----
            def inv_norm(ss, shape, tag, fold_temp):
                nrm = small.tile(shape, mybir.dt.float32, tag=tag + "_n")
                nc.scalar.sqrt(out=nrm[:], in_=ss[:])
                nc.vector.tensor_scalar(
                    out=nrm[:], in0=nrm[:], scalar1=float(EPS), scalar2=None,
                    op0=OP.max,
                )
                inv = small.tile(shape, mybir.dt.float32, tag=tag + "_i")
                nc.vector.reciprocal(out=inv[:], in_=nrm[:])
                if fold_temp:
                    nc.vector.tensor_scalar(
                        out=inv[:], in0=inv[:], scalar1=INV_TEMP, scalar2=None,
                        op0=OP.mult,
                    )
                return inv

            inva = inv_norm(ssa, [P, J, 1], "ia", fold_temp=True)  # has 1/temp
            invp = inv_norm(ssp, [P, J, 1], "ip", fold_temp=False)
            invg = inv_norm(ssg, [P, J, R], "ig", fold_temp=False)

            # ---- logits ----
            logit = small.tile([P, J, 1 + R], fp32, tag="logit")
            lp = logit[:, :, 0:1]
            lg = logit[:, :, 1 : 1 + R]
            nc.vector.tensor_tensor(out=lp, in0=dp[:], in1=inva[:], op=OP.mult)
            nc.vector.tensor_tensor(out=lp, in0=lp, in1=invp[:], op=OP.mult)
            nc.vector.tensor_tensor(
                out=lg, in0=dg[:], in1=inva[:].to_broadcast([P, J, R]), op=OP.mult
            )
            nc.vector.tensor_tensor(out=lg, in0=lg, in1=invg[:], op=OP.mult)

            # ---- 5-wide log-softmax:  nll = ln(sum(exp(l - m))) + m - lp ----
            mx = small.tile([P, J, 1], fp32, tag="mx")
            nc.vector.tensor_reduce(
                out=mx[:], in_=logit[:], axis=mybir.AxisListType.X, op=OP.max
            )
            lshift = small.tile([P, J, 1 + R], fp32, tag="lshift")
            nc.vector.tensor_tensor(
                out=lshift[:], in0=logit[:], in1=mx[:].to_broadcast([P, J, 1 + R]),
                op=OP.subtract,
            )
            ex = small.tile([P, J, 1 + R], fp32, tag="ex")
            nc.scalar.activation(out=ex[:], in_=lshift[:], func=AF.Exp)
            sume = small.tile([P, J, 1], fp32, tag="sume")
            nc.vector.tensor_reduce(
                out=sume[:], in_=ex[:], axis=mybir.AxisListType.X, op=OP.add
            )
            lns = small.tile([P, J, 1], fp32, tag="lns")
            nc.scalar.activation(out=lns[:], in_=sume[:], func=AF.Ln)
            nll = small.tile([P, J, 1], fp32, tag="nll")
            nc.vector.tensor_tensor(out=nll[:], in0=lns[:], in1=mx[:], op=OP.add)
            nc.vector.tensor_tensor(out=nll[:], in0=nll[:], in1=lp, op=OP.subtract)

            # ---- partial = sum over all 1024 rows (free-dim then partitions) ----
            rsum = small.tile([P, 1], fp32, tag="rsum")
            nc.vector.tensor_reduce(
                out=rsum[:], in_=nll[:], axis=mybir.AxisListType.XY, op=OP.add
            )
            ones = small.tile([P, 1], fp32, tag="ones")
            nc.vector.memset(ones[:], 1.0)
            psc = pp.tile([1, 1], fp32, tag="psc")
            nc.tensor.matmul(out=psc[:], lhsT=ones[:], rhs=rsum[:], start=True, stop=True)
            res = small.tile([1, 1], fp32, tag="res")
            nc.vector.tensor_copy(out=res[:], in_=psc[:])
            nc.sync.dma_start(out=out_partial[:], in_=res[:])
            if dbg:
                nc.sync.dma_start(out=out_logit[:], in_=logit[:])
                nc.sync.dma_start(out=out_cols[:], in_=cols[:])
                nc.sync.dma_start(
                    out=out_g[:], in_=g_all[:].rearrange("p r j d -> p (r j d)")
                )

    nc.finalize()  # runs Bacc.compile(): wait legalization + reg alloc
    return nc


def make_in_maps(z_i, z_j, neg_idx):
    z = np.ascontiguousarray(np.concatenate([z_i, z_j], axis=0), dtype=np.float32)
    neg_idx = np.asarray(neg_idx, dtype=np.int32)
    in_maps = []
    for m in range(NCORES):
        lo = m * RPC
        plo = (lo + B) % N
        # [RPC, R] -> [J, P, R] -> [P, R, J]
        ni = neg_idx[lo : lo + RPC].reshape(J, P, R).transpose(1, 2, 0)
        rows = np.arange(lo, lo + RPC, dtype=np.int32).reshape(J, P).T  # [P, J]
        idx = np.ascontiguousarray(
            np.concatenate([ni, rows[:, None, :]], axis=1)
        )
        zop = np.ascontiguousarray(
            np.concatenate([z[lo : lo + RPC], z[plo : plo + RPC]], axis=0)
        )
        in_maps.append({"z_full": z, "zop": zop, "idx": idx})
    return in_maps


def kernel(z_i, z_j, neg_idx, _bench=None):
    from concourse.bass_utils import run_bass_kernel_spmd

    if "nc" not in _CACHE:
        _CACHE["nc"] = build_nc()
    nc = _CACHE["nc"]
    in_maps = make_in_maps(z_i, z_j, neg_idx)
    core_ids = list(range(NCORES))
    kw = dict(_bench or {})
    r = run_bass_kernel_spmd(nc, in_maps, core_ids, **kw)
    if _bench is not None:
        _CACHE["last_results"] = r
    total = np.sum(
        [r.results[m]["partial"][0, 0] for m in range(NCORES)], dtype=np.float64
    )
    return np.float32(total / N)

